# revision 25
# baseline (speedup 1.0000x reference)
"""Multi-head attention (RoPE, causal) Trainium2 Bass kernel, 8-core SPMD.

Sharding: core c = (batch b = c // 4, head-group g = c % 4); each core computes
4 of the 16 heads for one batch, including its slice of the Q/K/V projections
and a partial output projection.  The host sums the 4 partial outputs per
batch (tensor-parallel unshard).

Device layout notes:
  - x is pre-transposed on host to xT [D, S] so projection matmuls contract
    over D on partitions.
  - Wq/Wk rows are host-permuted so the projection PSUM M-tiles are directly
    the RoPE operand layouts: tile0 = even ("x1") dims of all 4 heads
    stacked [h0(32) h1 h2 h3], tile1 = odd ("x2") dims.  RoPE is then pure
    lane-aligned elementwise DVE work producing qFH/qSH (kFH/kSH) tiles whose
    partition rows are 32-per-head stacks.
  - scores are computed transposed, scoresT[sk, sq], one PSUM bank per head
    (row strips 32h -> concurrent matmuls), softmax runs without max
    subtraction as exp(s/8 - 8) (exact: constant shift), the denominator is a
    col-packed M=1 ones-matmul per head, and AV needs no transposes:
    out_hT[dh, sq] = v_h[sk, dh].T @ expT[sk, sq].
  - causal structure is exploited generally: the host classifies every
    (sq-block 512 x sk-tile 128) mask block as skip / full / pattern and the
    kernel only emits work for non-skip blocks, narrowing columns to the
    non-masked range.  Patterns (0/1) multiply the exp tile - exact.
"""

import sys
import types

for _p in ("/opt/trn_rl_repo", "/root/.axon_site"):
    if _p not in sys.path:
        sys.path.insert(0, _p)

import numpy as np
import ml_dtypes

import concourse.bacc as bacc
import concourse.mybir as mybir
import concourse.tile as tile
from concourse.bass_utils import run_bass_kernel_spmd

BF16 = mybir.dt.bfloat16
F32 = mybir.dt.float32
NP_BF16 = ml_dtypes.bfloat16

# Problem constants (hardcoded per contract)
B, S, D = 2, 2048, 1024
H, DH = 16, 64
ROPE_BASE = 10000.0
NCORES = 8
GROUPS = 4            # head-groups per batch
HPC = H // GROUPS     # 4 heads per core
DC = HPC * DH         # 256 head dims per core
SB = 512              # sq block
NSB = S // SB         # 4 sq blocks
SK = 128              # sk tile
NSK = S // SK         # 16 sk tiles
DT = D // 128         # 8 di tiles
SCALE = 1.0 / np.sqrt(DH)
EXP_SHIFT = -8.0


# ---------------------------------------------------------------- host prep

def _rope_tables():
    """CE/SE/SO/CO [32, S] per reference's interleaved-rope formula,
    tiled x4 on partitions -> [4, 128, S] float32."""
    inv_freq = 1.0 / (ROPE_BASE ** (np.arange(0, DH, 2, dtype=np.float64) / DH))
    t = np.arange(S, dtype=np.float64)
    freqs = np.outer(t, inv_freq)                    # [S, 32]
    emb = np.concatenate([freqs, freqs], axis=-1)    # [S, 64]
    m = np.arange(32)
    ce = np.cos(emb[:, 2 * m]).T                     # [32, S]
    se = np.sin(emb[:, 2 * m]).T
    so = np.sin(emb[:, 2 * m + 1]).T
    co = np.cos(emb[:, 2 * m + 1]).T
    # packed for fused rope: T1=[CE;SO], T2=[-SE;CO] so FH/SH are one add
    out = np.stack([ce, so, -se, co]).astype(np.float32)   # [4, 32, S]
    return np.tile(out, (1, 4, 1))                   # [4, 128, S]


PATW = 128            # pattern tile width (c0-relative)


def _plan_mask(mask):
    """Classify each (sq-block, sk-tile) mask block.

    Returns (units, patterns): units[Bb] = list of (k, c0, c1, pidx|None);
    patterns = [128, PATW] 0/1 bf16 tiles (transposed blocks, c0-relative).
    Skip blocks are omitted.  Columns < c0 of a kept block are all-masked,
    columns >= c1 are all-allowed, and [c0, c1) multiplies pattern pidx."""
    units = []
    pat_idx = {}
    pats = []
    for Bb in range(NSB):
        row = []
        for k in range(NSK):
            bt = mask[SB * Bb:SB * (Bb + 1), SK * k:SK * (k + 1)].T  # [128sk, 512sq]
            bt = (bt != 0)
            any_col = bt.any(axis=0)
            if not any_col.any():
                continue
            all_col = bt.all(axis=0)
            c0 = int(np.argmax(any_col))
            not_all = np.nonzero(~all_col)[0]
            c1 = int(not_all.max()) + 1 if len(not_all) else 0
            c1 = max(c1, c0)
            pidx = None
            if c1 > c0:
                if c1 - c0 > PATW:
                    # wide partial region: fall back to multiple narrower
                    # patterns is not implemented; widen PATW would be needed
                    raise NotImplementedError("mask partial region > PATW")
                key = bt[:, c0:c1].tobytes()
                if key not in pat_idx:
                    pat_idx[key] = len(pats)
                    p = np.zeros((128, PATW), dtype=NP_BF16)
                    p[:, :c1 - c0] = bt[:, c0:c1].astype(NP_BF16)
                    pats.append(p)
                pidx = pat_idx[key]
            row.append((k, c0, c1, pidx))
        units.append(row)
    if not pats:
        pats.append(np.zeros((128, PATW), dtype=NP_BF16))
    return units, np.stack(pats)


def _prep_core_inputs(x, Wq, Wk, Wv, Wo, tables, patterns, core):
    b, g = core // GROUPS, core % GROUPS
    heads = [GROUPS * g + j for j in range(HPC)]

    xT = np.ascontiguousarray(x[b].T).astype(NP_BF16).reshape(DT, 128, S)

    x1_rows = [64 * h + 2 * m for h in heads for m in range(32)]
    x2_rows = [64 * h + 2 * m + 1 for h in heads for m in range(32)]
    wq = np.ascontiguousarray(Wq[x1_rows + x2_rows].T).astype(NP_BF16).reshape(DT, 128, DC)
    wk = np.ascontiguousarray(Wk[x1_rows + x2_rows].T).astype(NP_BF16).reshape(DT, 128, DC)

    v_rows = [64 * h + d for h in heads for d in range(DH)]
    wv = np.ascontiguousarray(Wv[v_rows].T).astype(NP_BF16).reshape(DT, 128, DC)

    # att_outT partition tiles hold local heads [0,2] and [1,3]
    wo_cols = [64 * heads[j] + d for j in (0, 2, 1, 3) for d in range(DH)]
    wo = np.ascontiguousarray(Wo[:, wo_cols].T).astype(NP_BF16).reshape(2, 128, D)

    # PE shuffle selectors: q2/k2 pair layouts from FH/SH-stacked rope output
    shuf = np.zeros((4, 128, 128), dtype=NP_BF16)
    for p_ in range(2):
        for k_ in range(32):
            base = 64 * p_
            shuf[2 * p_, base + k_, k_] = 1          # fh head 2p -> rows 0-31
            shuf[2 * p_, base + 32 + k_, 64 + k_] = 1  # fh head 2p+1 -> 64-95
            shuf[2 * p_ + 1, base + k_, 32 + k_] = 1   # sh head 2p -> 32-63
            shuf[2 * p_ + 1, base + 32 + k_, 96 + k_] = 1  # sh head 2p+1 -> 96-127
    return {
        "xT": xT, "wq": wq, "wk": wk, "wv": wv, "wo": wo,
        "rope": tables.astype(NP_BF16),
        "pats": patterns, "shuf": shuf,
    }


# ---------------------------------------------------------------- program

_CACHE = {}


def _build(units, npat):
    nc = bacc.Bacc(None)
    xT_d = nc.declare_dram_parameter("xT", [DT, 128, S], BF16, isOutput=False)
    wq_d = nc.declare_dram_parameter("wq", [DT, 128, DC], BF16, isOutput=False)
    wk_d = nc.declare_dram_parameter("wk", [DT, 128, DC], BF16, isOutput=False)
    wv_d = nc.declare_dram_parameter("wv", [DT, 128, DC], BF16, isOutput=False)
    wo_d = nc.declare_dram_parameter("wo", [2, 128, D], BF16, isOutput=False)
    rope_d = nc.declare_dram_parameter("rope", [4, 128, S], BF16, isOutput=False)
    pats_d = nc.declare_dram_parameter("pats", [npat, 128, PATW], BF16, isOutput=False)
    shuf_d = nc.declare_dram_parameter("shuf", [4, 128, 128], BF16, isOutput=False)
    y_d = nc.declare_dram_parameter("y", [DT, 128, S], BF16, isOutput=True)

    with tile.TileContext(nc) as tc:
        _emit(nc, tc, xT_d, wq_d, wk_d, wv_d, wo_d, rope_d, pats_d, shuf_d,
              y_d, units, npat)
    nc.compile()
    return nc


def _emit(nc, tc, xT_d, wq_d, wk_d, wv_d, wo_d, rope_d, pats_d, shuf_d,
          y_d, units, npat):
    from contextlib import ExitStack
    ctx = ExitStack()
    with ctx:
        const = ctx.enter_context(tc.tile_pool(name="const", bufs=1))
        persist = ctx.enter_context(tc.tile_pool(name="persist", bufs=1))
        work = ctx.enter_context(tc.tile_pool(name="work", bufs=4))
        xp = ctx.enter_context(tc.tile_pool(name="xp", bufs=2))
        normc = ctx.enter_context(tc.tile_pool(name="normc", bufs=2))
        expp = ctx.enter_context(tc.tile_pool(name="expp", bufs=8))
        yp = ctx.enter_context(tc.tile_pool(name="yp", bufs=2))
        # PSUM: pair tiles [128,2,SB] x3 (scores + proj filler) + 2 AV banks
        psA = ctx.enter_context(tc.tile_pool(name="psA", bufs=3, space="PSUM"))
        psAV = ctx.enter_context(tc.tile_pool(name="psAV", bufs=2, space="PSUM"))

        # ---- weights first (critical path at startup), big tables later.
        # Constant loads go on the GpSimd queue (ordered by first use) so xT
        # loads on the Sync queue are not stuck behind their ~1us-per-DMA
        # issue overhead.
        # Weights go on the GpSimd queue (~1.8us issue overhead per DMA);
        # small tables ride the cheaper Sync queue between the xT loads.
        h8 = DT // 2
        wq_s = persist.tile([128, DT, DC], BF16, tag="wq")
        nc.gpsimd.dma_start(wq_s[:, :h8, :],
                            wq_d[:h8].rearrange("t p c -> p t c"))
        nc.gpsimd.dma_start(wq_s[:, h8:, :],
                            wq_d[h8:].rearrange("t p c -> p t c"))
        wk_s = persist.tile([128, DT, DC], BF16, tag="wk")
        nc.gpsimd.dma_start(wk_s[:], wk_d.rearrange("t p c -> p t c"))
        ropes = const.tile([128, 4, S], BF16, tag="rope")
        # ropes layout: [CE, SO, -SE, CO] -> T1 = [:,0:2], T2 = [:,2:4]
        nc.gpsimd.dma_start(ropes[:], rope_d.rearrange("t p s -> p t s"))
        shuf_s = const.tile([128, 4, 128], BF16, tag="shuf")
        nc.gpsimd.dma_start(shuf_s[:], shuf_d.rearrange("n p m -> p n m"))
        pats = const.tile([128, npat, PATW], BF16, tag="pats")
        nc.gpsimd.dma_start(pats[:], pats_d.rearrange("n p s -> p n s"))
        wv_s = persist.tile([128, DT, DC], BF16, tag="wv")
        nc.gpsimd.dma_start(wv_s[:], wv_d.rearrange("t p c -> p t c"))
        wo_s = persist.tile([128, 2, D], BF16, tag="wo")
        nc.gpsimd.dma_start(wo_s[:], wo_d.rearrange("t p c -> p t c"))
        bias8 = const.tile([128, 1], F32, tag="bias8")
        nc.vector.memset(bias8[:], EXP_SHIFT)
        # warm the Exp activation table while startup DMAs are in flight
        wsrc = const.tile([128, 8], F32, tag="wsrc")
        nc.vector.memset(wsrc[:], 0.0)
        warm = const.tile([128, 8], BF16, tag="warm")
        nc.scalar.activation(warm[:], wsrc[:],
                             mybir.ActivationFunctionType.Exp,
                             bias=bias8[:], scale=SCALE)

        # ---- persistent per-B activations
        vSB, attT, q2, k2 = {}, {}, {}, {}
        for Bb in range(NSB):
            vSB[Bb] = persist.tile([128, 4, HPC, 65], BF16, tag=f"v{Bb}",
                                   name=f"v{Bb}")  # 4 sk tiles, per-head [v|1]
            attT[Bb] = persist.tile([128, 2, SB], BF16, tag=f"att{Bb}",
                                    name=f"att{Bb}")
            q2[Bb] = persist.tile([128, 2, SB], BF16, tag=f"q2{Bb}", name=f"q2{Bb}")
            k2[Bb] = persist.tile([128, 2, SB], BF16, tag=f"k2{Bb}", name=f"k2{Bb}")

        def kv_tiles(k):          # global sk tile -> (block idx, col offset)
            return k // 4, (k % 4) * SK

        def load_x(Bb):
            s0 = SB * Bb
            xT = xp.tile([128, DT, SB], BF16, tag="xT", name=f"xT{Bb}")
            h = DT // 2
            nc.sync.dma_start(xT[:, :h, :],
                              xT_d[:h, :, s0:s0 + SB].rearrange("t p s -> p t s"))
            nc.sync.dma_start(xT[:, h:, :],
                              xT_d[h:, :, s0:s0 + SB].rearrange("t p s -> p t s"))
            return xT

        def proj_pieces(Bb, xT, startup=False):
            """Generator of emit-closures: q/k/v projections + rope + pair
            shuffle for block Bb, sliced into PE-filler sized pieces."""
            s0 = SB * Bb
            ps_qk = {}

            def qk_chain(w_s, nm, mt, lo, hi):
                def go():
                    if nm not in ps_qk:
                        ps_qk[nm] = psA.tile([128, 2, SB], F32, tag="ps",
                                             name=f"{nm}ps{Bb}")
                    ps = ps_qk[nm]
                    for dt_i in range(lo, hi):
                        nc.tensor.matmul(
                            ps[:, mt, :], w_s[:, dt_i, 128 * mt:128 * (mt + 1)],
                            xT[:, dt_i, :],
                            start=(dt_i == 0), stop=(dt_i == DT - 1))
                return go

            rope_t = {}
            ps_sh = {}

            def rope_piece(nm, lo, hi):
                def go():
                    ps = ps_qk[nm]
                    if nm not in rope_t:
                        xb = work.tile([128, 2, SB], BF16, tag="ropein",
                                       name=f"{nm}xb{Bb}")
                        ta = work.tile([128, 2, SB], BF16, tag="ropea",
                                       name=f"{nm}ta{Bb}")
                        tb = work.tile([128, 2, SB], BF16, tag="ropeb",
                                       name=f"{nm}tb{Bb}")
                        fs = work.tile([128, 2, SB], BF16, tag="ropef",
                                       name=f"{nm}fs{Bb}")
                        rope_t[nm] = (xb, ta, tb, fs)
                    xb, ta, tb, fs = rope_t[nm]
                    w = hi - lo
                    nc.vector.tensor_copy(xb[:, :, lo:hi], ps[:, :, lo:hi])
                    nc.vector.tensor_mul(
                        ta[:, :, lo:hi],
                        xb[:, 0:1, lo:hi].to_broadcast([128, 2, w]),
                        ropes[:, 0:2, s0 + lo:s0 + hi])
                    nc.vector.tensor_mul(
                        tb[:, :, lo:hi],
                        xb[:, 1:2, lo:hi].to_broadcast([128, 2, w]),
                        ropes[:, 2:4, s0 + lo:s0 + hi])
                    nc.vector.tensor_add(fs[:, :, lo:hi], ta[:, :, lo:hi],
                                         tb[:, :, lo:hi])
                return go

            def shuf_piece(nm, lo, hi):
                def go():
                    fs = rope_t[nm][3]
                    # pair shuffle on PE -> [fh;sh]-per-head K=64 layout
                    t2 = q2[Bb] if nm == "q" else k2[Bb]
                    if nm not in ps_sh:
                        ps_sh[nm] = psA.tile([128, 2, SB], F32, tag="ps",
                                             name=f"{nm}s2{Bb}")
                    ps2 = ps_sh[nm]
                    for p_ in range(2):
                        nc.tensor.matmul(ps2[:, p_, lo:hi], shuf_s[:, 2 * p_, :],
                                         fs[:, 0, lo:hi], start=True, stop=False)
                        nc.tensor.matmul(ps2[:, p_, lo:hi],
                                         shuf_s[:, 2 * p_ + 1, :],
                                         fs[:, 1, lo:hi], start=False, stop=True)
                    nc.vector.tensor_copy(t2[:, :, lo:hi], ps2[:, :, lo:hi])
                return go

            ps_v = {}

            def v_piece(pp, lo, hi):
                def go():
                    if pp not in ps_v:
                        ps_v[pp] = psA.tile([128, 2, SB], F32, tag="ps",
                                            name=f"vps{Bb}_{pp}")
                    ps = ps_v[pp]
                    for half in range(2):
                        ck = 2 * pp + half
                        for dt_i in range(lo, hi):
                            nc.tensor.matmul(
                                ps[:, half, :DC],
                                xT[:, dt_i, 128 * ck:128 * (ck + 1)],
                                wv_s[:, dt_i, :],
                                start=(dt_i == 0), stop=(dt_i == DT - 1))
                    if hi < DT:
                        return
                    for half in range(2):
                        ck = 2 * pp + half
                        nc.vector.tensor_copy(
                            vSB[Bb][:, ck, :, 0:64],
                            ps[:, half, :DC].rearrange(
                                "p (j d) -> p j d", j=HPC))
                    if pp == 1:
                        nc.vector.memset(vSB[Bb][:, :, :, 64:65], 1.0)
                return go

            HC = SB // 2
            if startup:
                h = DT // 2
                for mt in range(2):
                    yield qk_chain(wq_s, "q", mt, 0, h)
                    yield qk_chain(wq_s, "q", mt, h, DT)
            else:
                yield qk_chain(wq_s, "q", 0, 0, DT)
                yield qk_chain(wq_s, "q", 1, 0, DT)
            yield rope_piece("q", 0, HC)
            yield shuf_piece("q", 0, HC)
            yield rope_piece("q", HC, SB)
            yield shuf_piece("q", HC, SB)
            yield qk_chain(wk_s, "k", 0, 0, DT)
            yield qk_chain(wk_s, "k", 1, 0, DT)
            yield rope_piece("k", 0, HC)
            yield shuf_piece("k", 0, HC)
            yield rope_piece("k", HC, SB)
            yield shuf_piece("k", HC, SB)
            yield v_piece(0, 0, DT)
            yield v_piece(1, 0, DT)

        def wo_pieces(Bb):
            s0 = SB * Bb
            ybig = yp.tile([128, DT, SB], BF16, tag="y", name=f"y{Bb}")

            def pair(pp):
                def go():
                    ps = psA.tile([128, 2, SB], F32, tag="ps",
                                  name=f"yps{Bb}_{pp}")
                    for half in range(2):
                        t = 2 * pp + half
                        for c in range(2):
                            nc.tensor.matmul(
                                ps[:, half, :], wo_s[:, c, 128 * t:128 * (t + 1)],
                                attT[Bb][:, c, :],
                                start=(c == 0), stop=(c == 1))
                    nc.vector.tensor_copy(ybig[:, 2 * pp:2 * pp + 2, :], ps[:])
                    if pp % 2 == 1:                # drain y half-block early
                        h4 = 2 * (pp - 1)
                        nc.sync.dma_start(
                            y_d[h4:h4 + 4, :, s0:s0 + SB].rearrange(
                                "t p s -> p t s"),
                            ybig[:, h4:h4 + 4, :])
                return go
            return [pair(pp) for pp in range(DT // 2)]

        def att_block(Bb, filler=()):
            filler = list(filler)
            row = units[Bb]
            if not row:
                nc.vector.memset(attT[Bb][:], 0.0)
                for f in filler:
                    f()
                return
            # distribute filler pieces across the two passes' unit slots
            slots = 2 * len(row)
            nf = max(1, slots // max(1, len(filler))) if filler else 0
            slot = 0

            for p in range(2):                     # head-pair pass
                av = [psAV.tile([65, SB], F32, tag="av", name=f"av{Bb}_{p}_{h}")
                      for h in range(2)]

                def emit_scores(ui):
                    k, c0, c1, pidx = row[ui]
                    kb, ko = kv_tiles(k)
                    sc = psA.tile([128, 2, SB], F32, tag="ps",
                                  name=f"sc{Bb}_{p}_{ui}")
                    for half in range(2):
                        nc.tensor.matmul(
                            sc[:, half, c0:],
                            k2[kb][64 * half:64 * (half + 1), p, ko:ko + SK],
                            q2[Bb][64 * half:64 * (half + 1), p, c0:],
                            start=True, stop=True,
                            tile_position=(64 * half, 0))
                    ex = expp.tile([128, 2, SB], BF16, tag="exp",
                                   name=f"ex{Bb}_{p}_{ui}")
                    if ui == 0 and c0 > 0:
                        nc.vector.memset(ex[:], 0.0)
                    nc.scalar.activation(
                        ex[:, :, c0:], sc[:, :, c0:],
                        mybir.ActivationFunctionType.Exp,
                        bias=bias8[:], scale=SCALE)
                    if pidx is not None:
                        nc.vector.tensor_mul(
                            ex[:, :, c0:c1], ex[:, :, c0:c1],
                            pats[:, pidx:pidx + 1, 0:c1 - c0].to_broadcast(
                                [128, 2, c1 - c0]))
                    return ex

                def emit_av(ui, ex):
                    k, c0, c1, pidx = row[ui]
                    kb, ko = kv_tiles(k)
                    first, last = ui == 0, ui == len(row) - 1
                    w0 = 0 if first else c0        # accum write start col
                    for half in range(2):
                        j = 2 * p + half
                        nc.tensor.matmul(
                            av[half][:, w0:],
                            vSB[kb][:, ko // SK, j, :],
                            ex[:, half, w0:],
                            start=first, stop=last)

                pend = []
                for ui in range(len(row)):
                    ex = emit_scores(ui)
                    pend.append((ui, ex))
                    # lag-4: the first AV of a pass waits for the previous
                    # pass's av-bank frees (normalize chain); the in-order PE
                    # queue stalls behind it unless scores+fillers run first.
                    if len(pend) > 4:
                        emit_av(*pend.pop(0))
                    if filler and nf and slot % nf == nf - 1:
                        f = filler.pop(0) if filler else None
                        if f:
                            f()
                    slot += 1
                while pend:
                    emit_av(*pend.pop(0))

                # normalize this pass (heads 2p, 2p+1) straight out of PSUM
                dget = normc.tile([1, 2 * SB], F32, tag="dget",
                                  name=f"dget{Bb}_{p}")
                for half in range(2):
                    nc.vector.tensor_copy(dget[:, SB * half:SB * (half + 1)],
                                          av[half][64:65, :])
                rc = normc.tile([1, 2 * SB], F32, tag="recip", name=f"rc{Bb}_{p}")
                nc.vector.reciprocal_approx_fast(out=rc[:], in_=dget[:])
                bc = normc.tile([64, 2 * SB], F32, tag="bc", name=f"bc{Bb}_{p}")
                nc.gpsimd.partition_broadcast(bc[:], rc[0:1, :])
                for half in range(2):
                    j = 2 * p + half               # head j -> ptile j%2, rows 64*(j//2)
                    bank, rhalf = j % 2, j // 2
                    nc.vector.tensor_mul(
                        attT[Bb][64 * rhalf:64 * (rhalf + 1), bank, :],
                        av[half][0:64, :], bc[:, SB * half:SB * (half + 1)])
            for f in filler:
                f()

        # ---- schedule: proj(B+1) fills att(B); all Wo deferred to att(3),
        # which is otherwise scalar(exp)-bound while PE idles.
        xT0 = load_x(0)
        for f in proj_pieces(0, xT0, startup=True):
            f()
        xT1 = load_x(1)
        att_block(0, list(proj_pieces(1, xT1)))
        xT2 = load_x(2)
        att_block(1, list(proj_pieces(2, xT2)))
        xT3 = load_x(3)
        att_block(2, list(proj_pieces(3, xT3)))
        att_block(3, wo_pieces(0) + wo_pieces(1) + wo_pieces(2))
        for f in wo_pieces(3):
            f()


# ---------------------------------------------------------------- entry

def _get_program(mask):
    key = mask.tobytes()
    if key not in _CACHE:
        units, patterns = _plan_mask(np.asarray(mask))
        nc = _build(units, patterns.shape[0])
        _CACHE[key] = (nc, units, patterns)
    return _CACHE[key]


def kernel(x, Wq, Wk, Wv, Wo, attn_mask, _trace=False):
    x = np.asarray(x, dtype=np.float32)
    Wq, Wk, Wv, Wo = (np.asarray(w, dtype=np.float32) for w in (Wq, Wk, Wv, Wo))
    attn_mask = np.asarray(attn_mask)

    nc, units, patterns = _get_program(attn_mask)
    tables = _rope_tables()
    in_maps = [_prep_core_inputs(x, Wq, Wk, Wv, Wo, tables, patterns, c)
               for c in range(NCORES)]
    res = run_bass_kernel_spmd(nc, in_maps, core_ids=list(range(NCORES)),
                               trace=_trace)

    out = np.zeros((B, S, D), dtype=np.float32)
    for c in range(NCORES):
        yT = res.results[c]["y"].astype(np.float32).reshape(D, S)
        out[c // GROUPS] += yT.T
    if _trace:
        return out, res
    return out

